# revision 8
# baseline (speedup 1.0000x reference)
"""ActorCriticLoss (TD-lambda + symlog critic) on 8 Trainium2 NeuronCores.

Data-parallel: batch axis (65536) sharded 8 ways; each core reduces its
(8192, 64) shard to per-partition partial sums/extrema; the O(1) loss
assembly runs on the host in float64.

Math: with phi_t = ret_t + (K1/K2)*v_t the TD(lambda) recurrence becomes
  phi_t = (r_t + (K1/K2) v_t) + K2 c_t phi_{t+1}
which cancels the c*v_next product of the naive form. The backward scan
runs as one forward `tensor_tensor_scan` per tile over per-row padded,
time-reversed streams ([pad, t=63..0] per row, k_pad=0,
a_pad=bootstrap*(1+K1/K2)), so the fp32 carry reinitializes at every row
boundary. We then use retm = (K1/K2)*v - phi = -ret everywhere and fix
signs on the host (max(retm) = -min(ret), sum lp*retm = -sum lp*ret,
sign(retm) = -sign(ret)).

Engine assignment (measured per-instruction HW costs):
  DVE : a-stream, scan, retm, sum(lp*retm), sign-copies (symlog), min, max
  ACT : k-stream (scaled copy), |v|, ln(1+|v|), |retm|, ln(1+|retm|),
        sum((symlog v - symlog ret)^2) via Square+accum
  Pool: lp*v product, d = sv + sr', pads
  PE  : ones-matmul reductions of lp, entropy, lp*v into PSUM
"""

import sys

import numpy as np

sys.path.insert(0, "/opt/trn_rl_repo")

import concourse.bass as bass  # noqa: E402
import concourse.mybir as mybir  # noqa: E402
import concourse.tile as tile  # noqa: E402
from concourse import bacc  # noqa: E402
from concourse.bass_utils import run_bass_kernel_spmd  # noqa: E402

B, T = 65536, 64
NCORES = 8
B_LOC = B // NCORES          # 8192 rows per core
P = 128
M = 16                       # rows per partition per tile
NT = B_LOC // (P * M)        # tiles per core
F = M * T                    # elements/partition per tile
S = T + 1
FP = M * S

DISCOUNT, LAMBDA = 0.997, 0.95
ENTROPY_SCALE = 0.0003
RETURN_EMA_DECAY = 0.99
K2 = DISCOUNT * LAMBDA
RATIO = (1.0 - LAMBDA) / LAMBDA
SIGN_MASK = 0x80000000

f32 = mybir.dt.float32
u32 = mybir.dt.uint32
AX = mybir.AxisListType
OP = mybir.AluOpType
AF = mybir.ActivationFunctionType

# acc_dve columns: u1m(0:NT) mxm(NT:2NT) mnm(2NT:3NT); acc_act: d2(0:NT)
N_DVE = 3 * NT
N_ACT = NT
N_OUT = N_DVE + N_ACT
PE_N = 512  # matmul free-dim chunk (PSUM bank limit)


def _stt_uint_imm(eng, out, in0, imm, in1, op0, op1, accum_out=None):
    """scalar_tensor_tensor with a uint32-typed immediate (the public
    wrapper emits float32 immediates; bitvec ops need integer-typed)."""
    outs = [eng.lower_ap(out)]
    if accum_out is not None:
        outs.append(eng.lower_ap(accum_out))
    return eng.add_instruction(
        mybir.InstTensorScalarPtr(
            name=eng.bass.get_next_instruction_name(),
            is_scalar_tensor_tensor=True,
            op0=op0,
            op1=op1,
            ins=[
                eng.lower_ap(in0),
                mybir.ImmediateValue(dtype=u32, value=imm),
                eng.lower_ap(in1),
            ],
            outs=outs,
        )
    )


def build_module():
    nc = bacc.Bacc(
        "TRN2", target_bir_lowering=False, debug=False, enable_asserts=False
    )
    r_d = nc.dram_tensor("rewards", [B_LOC, T], f32, kind="ExternalInput").ap()
    v_d = nc.dram_tensor("values", [B_LOC, T], f32, kind="ExternalInput").ap()
    c_d = nc.dram_tensor("continues", [B_LOC, T], f32, kind="ExternalInput").ap()
    bs_d = nc.dram_tensor("bootstrap", [B_LOC], f32, kind="ExternalInput").ap()
    lp_d = nc.dram_tensor("log_probs", [B_LOC, T], f32, kind="ExternalInput").ap()
    en_d = nc.dram_tensor("entropy", [B_LOC, T], f32, kind="ExternalInput").ap()
    out_d = nc.dram_tensor("out", [P, N_OUT], f32, kind="ExternalOutput").ap()
    pe_d = nc.dram_tensor("pe_out", [1, 3 * PE_N], f32, kind="ExternalOutput").ap()

    r_v = r_d.rearrange("(n p m) t -> n p (m t)", p=P, m=M)
    v_v = v_d.rearrange("(n p m) t -> n p (m t)", p=P, m=M)
    c_v = c_d.rearrange("(n p m) t -> n p (m t)", p=P, m=M)
    lp_v = lp_d.rearrange("(n p m) t -> n p (m t)", p=P, m=M)
    en_v = en_d.rearrange("(n p m) t -> n p (m t)", p=P, m=M)
    bs_v = bs_d.rearrange("(n p m) -> p n m", p=P, m=M)

    with tile.TileContext(nc) as tc:
        with (
            tc.tile_pool(name="const", bufs=1) as constp,
            tc.tile_pool(name="ins", bufs=2) as ins,
            tc.tile_pool(name="work", bufs=2) as work,
            tc.tile_pool(name="accp", bufs=1) as accp,
            tc.tile_pool(name="psum", bufs=1, space="PSUM") as psp,
        ):
            acc_dve = accp.tile([P, N_DVE], f32)
            acc_act = accp.tile([P, N_ACT], f32)
            ones = constp.tile([P, 1], f32)
            nc.gpsimd.memset(ones[:], 1.0)
            bs_all = constp.tile([P, B_LOC // P], f32)  # [128, NT*M]
            nc.sync.dma_start(
                bs_all[:].rearrange("p (n m) -> p n m", m=M), bs_v
            )
            ps_lp = psp.tile([1, PE_N], f32)
            ps_en = psp.tile([1, PE_N], f32)
            ps_u2 = psp.tile([1, PE_N], f32)

            for n in range(NT):
                r_t = ins.tile([P, F], f32)
                v_t = ins.tile([P, F], f32)
                c_t = ins.tile([P, F], f32)
                lp_t = ins.tile([P, F], f32)
                en_t = ins.tile([P, F], f32)
                nc.sync.dma_start(r_t[:], r_v[n])
                nc.sync.dma_start(v_t[:], v_v[n])
                nc.sync.dma_start(c_t[:], c_v[n])
                nc.sync.dma_start(lp_t[:], lp_v[n])
                nc.sync.dma_start(en_t[:], en_v[n])

                a_t = work.tile([P, FP], f32)
                k_t = work.tile([P, FP], f32)
                phi_t = work.tile([P, FP], f32)
                retm_t = work.tile([P, F], f32)
                av_t = work.tile([P, F], f32)
                lnv_t = work.tile([P, F], f32)
                ar_t = work.tile([P, F], f32)
                lnr_t = work.tile([P, F], f32)
                d_t = work.tile([P, F], f32)
                j1_t = work.tile([P, F], f32)
                j2_t = work.tile([P, F], f32)

                r3 = r_t[:].rearrange("p (m t) -> p m t", t=T)
                v3 = v_t[:].rearrange("p (m t) -> p m t", t=T)
                c3 = c_t[:].rearrange("p (m t) -> p m t", t=T)
                a3 = a_t[:].rearrange("p (m s) -> p m s", s=S)
                k3 = k_t[:].rearrange("p (m s) -> p m s", s=S)
                phi3 = phi_t[:].rearrange("p (m s) -> p m s", s=S)
                retm3 = retm_t[:].rearrange("p (m t) -> p m t", t=T)
                a_rev = a3[:, :, 1:S][:, :, ::-1]
                k_rev = k3[:, :, 1:S][:, :, ::-1]
                phi_nat = phi3[:, :, T:0:-1]
                bs_n = bs_all[:, n * M : (n + 1) * M]

                # DVE: a = RATIO*v + r into reversed padded stream;
                #      pad = bootstrap * (1 + RATIO)
                nc.vector.scalar_tensor_tensor(
                    a_rev, v3, RATIO, r3, OP.mult, OP.add
                )
                nc.vector.tensor_scalar_mul(
                    a3[:, :, 0:1], bs_n.unsqueeze(2), 1.0 + RATIO
                )
                # ACT: k = K2*c reversed; Pool: k pad = 0
                nc.scalar.activation(k_rev, c3, AF.Copy, scale=K2)
                nc.gpsimd.memset(k3[:, :, 0:1], 0.0)
                # DVE: the TD(lambda) scan (one instruction per tile)
                nc.vector.tensor_tensor_scan(
                    phi_t[:], k_t[:], a_t[:], 0.0, OP.mult, OP.add
                )
                # DVE: retm = RATIO*v - phi  (= -ret)
                nc.vector.scalar_tensor_tensor(
                    retm3, v3, RATIO, phi_nat, OP.mult, OP.subtract
                )
                # DVE: extrema of retm (signs fixed on host)
                nc.vector.tensor_reduce(
                    acc_dve[:, NT + n : NT + n + 1], retm_t[:],
                    axis=AX.X, op=OP.max,
                )
                nc.vector.tensor_reduce(
                    acc_dve[:, 2 * NT + n : 2 * NT + n + 1], retm_t[:],
                    axis=AX.X, op=OP.min,
                )
                # DVE: sum(lp * retm)
                nc.vector.scalar_tensor_tensor(
                    j1_t[:], lp_t[:], 1.0, retm_t[:], OP.mult, OP.mult,
                    accum_out=acc_dve[:, n : n + 1],
                )
                # Pool: lp*v product; PE: its column-sum (and lp, entropy)
                nc.gpsimd.tensor_mul(j2_t[:], lp_t[:], v_t[:])
                for h in range(F // PE_N):
                    first = n == 0 and h == 0
                    last = n == NT - 1 and h == F // PE_N - 1
                    sl = slice(h * PE_N, (h + 1) * PE_N)
                    nc.tensor.matmul(
                        ps_lp[:], ones[:], lp_t[:, sl], start=first, stop=last
                    )
                    nc.tensor.matmul(
                        ps_en[:], ones[:], en_t[:, sl], start=first, stop=last
                    )
                    nc.tensor.matmul(
                        ps_u2[:], ones[:], j2_t[:, sl], start=first, stop=last
                    )
                # ACT: symlog magnitudes
                nc.scalar.activation(av_t[:], v_t[:], AF.Abs)
                nc.scalar.activation(lnv_t[:], av_t[:], AF.Ln, bias=1.0)
                nc.scalar.activation(ar_t[:], retm_t[:], AF.Abs)
                nc.scalar.activation(lnr_t[:], ar_t[:], AF.Ln, bias=1.0)
                # DVE: signed symlogs; sv = copysign(Lv, v),
                # sr' = copysign(Lr, retm) = -symlog(ret)
                _stt_uint_imm(
                    nc.vector, av_t[:].bitcast(u32), v_t[:].bitcast(u32),
                    SIGN_MASK, lnv_t[:].bitcast(u32),
                    OP.bitwise_and, OP.bitwise_or,
                )
                _stt_uint_imm(
                    nc.vector, ar_t[:].bitcast(u32), retm_t[:].bitcast(u32),
                    SIGN_MASK, lnr_t[:].bitcast(u32),
                    OP.bitwise_and, OP.bitwise_or,
                )
                # Pool: d = sv + sr' (= symlog v - symlog ret)
                nc.gpsimd.tensor_add(d_t[:], av_t[:], ar_t[:])
                # ACT: critic partial sums
                nc.scalar.activation(
                    j1_t[:], d_t[:], AF.Square,
                    accum_out=acc_act[:, n : n + 1],
                )

            pe_sb = accp.tile([1, 3 * PE_N], f32)
            nc.scalar.copy(pe_sb[:, 0:PE_N], ps_lp[:])
            nc.scalar.copy(pe_sb[:, PE_N : 2 * PE_N], ps_en[:])
            nc.scalar.copy(pe_sb[:, 2 * PE_N :], ps_u2[:])
            nc.sync.dma_start(out_d[:, 0:N_DVE], acc_dve[:])
            nc.sync.dma_start(out_d[:, N_DVE:N_OUT], acc_act[:])
            nc.sync.dma_start(pe_d, pe_sb[:])

    nc.compile()
    return nc


_NC = None


def _get_nc():
    global _NC
    if _NC is None:
        _NC = build_module()
    return _NC


def _run(in_maps, trace=False, **kwargs):
    return run_bass_kernel_spmd(
        _get_nc(), in_maps, core_ids=list(range(NCORES)), trace=trace, **kwargs
    )


def make_in_maps(rewards, values, continues, bootstrap, log_probs, entropy):
    in_maps = []
    for i in range(NCORES):
        sl = slice(i * B_LOC, (i + 1) * B_LOC)
        in_maps.append(
            {
                "rewards": np.ascontiguousarray(rewards[sl], dtype=np.float32),
                "values": np.ascontiguousarray(values[sl], dtype=np.float32),
                "continues": np.ascontiguousarray(continues[sl], dtype=np.float32),
                "bootstrap": np.ascontiguousarray(bootstrap[sl], dtype=np.float32),
                "log_probs": np.ascontiguousarray(log_probs[sl], dtype=np.float32),
                "entropy": np.ascontiguousarray(entropy[sl], dtype=np.float32),
            }
        )
    return in_maps


def combine(results):
    """Host-side O(1) finish. retm = -ret sign fixups happen here."""
    outs = np.stack([res["out"] for res in results]).astype(np.float64)
    pe = np.stack([res["pe_out"] for res in results]).astype(np.float64)
    u1 = -outs[:, :, 0:NT].sum()                 # sum lp*ret
    mn = -outs[:, :, NT : 2 * NT].max()          # min ret = -max(retm)
    mx = -outs[:, :, 2 * NT : 3 * NT].min()      # max ret = -min(retm)
    d2 = outs[:, :, N_DVE:].sum()
    slp = pe[:, 0, 0:PE_N].sum()
    sent = pe[:, 0, PE_N : 2 * PE_N].sum()
    u2 = pe[:, 0, 2 * PE_N :].sum()

    n = float(B * T)
    ema = 1.0 - RETURN_EMA_DECAY
    lo_n = ema * mn
    hi_n = 1.0 + ema * (mx - 1.0)
    scale = max(hi_n - lo_n, 1.0)
    pg = -((u1 / n) / scale - lo_n * (slp / n) / scale - (u2 / n))
    entropy_loss = -ENTROPY_SCALE * (sent / n)
    critic = d2 / n
    return np.float32(pg + entropy_loss + critic)


def kernel(rewards, values, continues, bootstrap, log_probs, entropy):
    in_maps = make_in_maps(
        rewards, values, continues, bootstrap, log_probs, entropy
    )
    results = _run(in_maps).results
    return combine(results)


# revision 9
# speedup vs baseline: 1.6623x; 1.6623x over previous
"""ActorCriticLoss (TD-lambda + symlog critic) on 8 Trainium2 NeuronCores.

Data-parallel over the batch axis (65536 -> 8 x 8192). The device reduces
each shard to per-partition partials; the O(1) loss assembly runs on the
host in float64.

Math: with phi_t = ret_t + (K1/K2) v_t the TD(lambda) recurrence becomes
  phi_t = a_t + K2 c_t phi_{t+1},   a_t = r_t + (K1/K2) v_t
(the c*v_next product cancels). The backward scan runs as one forward
`tensor_tensor_scan` per tile over per-row padded, time-reversed streams
([pad, t=63..0] per row, k_pad = 0, a_pad = bootstrap*(1+K1/K2)) so the
fp32 scan carry reinitializes at every row boundary. The device works with
retm = -ret throughout; signs are fixed on the host.

Split of labor (driven by measured per-instruction HW costs):
 - HOST (numpy, exact fp32/f64): builds the padded bf16 a/k streams and
   time-reversed bf16 v/log_probs; computes sum(entropy) and sum(lp*v)
   from the fp32 originals (entropy/rewards/continues never hit the device
   elementwise paths they aren't needed in).
 - DVE (all-bf16, 2x rate; fp32 scan carry + fp32 accumulators): scan,
   retm, sum(lp*retm), sign-copies via uint16 bit ops, d, min/max.
 - ACT: |v|, ln(1+|v|), |ret|, ln(1+|ret|), v->bf16, sum(d^2) via
   Square+accum (bf16 in, fp32 accumulate).
 - PE: sum(lp) via ones-matmul into PSUM.
"""

import sys

import ml_dtypes
import numpy as np

sys.path.insert(0, "/opt/trn_rl_repo")

import concourse.bass as bass  # noqa: E402
import concourse.mybir as mybir  # noqa: E402
import concourse.tile as tile  # noqa: E402
from concourse import bacc  # noqa: E402
from concourse.bass_utils import run_bass_kernel_spmd  # noqa: E402

B, T = 65536, 64
NCORES = 8
B_LOC = B // NCORES
P = 128
M = 16                       # rows per partition per tile
NT = B_LOC // (P * M)
F = M * T                    # payload elements/partition per tile
S = T + 1                    # padded slots per row
FP = M * S

DISCOUNT, LAMBDA = 0.997, 0.95
ENTROPY_SCALE = 0.0003
RETURN_EMA_DECAY = 0.99
K2 = DISCOUNT * LAMBDA
RATIO = (1.0 - LAMBDA) / LAMBDA

f32 = mybir.dt.float32
bf16 = mybir.dt.bfloat16
u16 = mybir.dt.uint16
AX = mybir.AxisListType
OP = mybir.AluOpType
AF = mybir.ActivationFunctionType
BF = ml_dtypes.bfloat16

# acc_dve columns: u1m(0:NT) mxm(NT:2NT) mnm(2NT:3NT); acc_act: d2(0:NT)
N_DVE = 3 * NT
N_ACT = NT
N_OUT = N_DVE + N_ACT
PE_N = 512


def _stt_uint_imm(eng, out, in0, imm, in1, op0, op1, imm_dtype=u16,
                  accum_out=None):
    """scalar_tensor_tensor with an integer-typed immediate (the public
    wrapper emits float32 immediates; bitvec ops need the immediate typed
    like src/dst)."""
    outs = [eng.lower_ap(out)]
    if accum_out is not None:
        outs.append(eng.lower_ap(accum_out))
    return eng.add_instruction(
        mybir.InstTensorScalarPtr(
            name=eng.bass.get_next_instruction_name(),
            is_scalar_tensor_tensor=True,
            op0=op0,
            op1=op1,
            ins=[
                eng.lower_ap(in0),
                mybir.ImmediateValue(dtype=imm_dtype, value=imm),
                eng.lower_ap(in1),
            ],
            outs=outs,
        )
    )


def build_module():
    nc = bacc.Bacc(
        "TRN2", target_bir_lowering=False, debug=False, enable_asserts=False
    )
    a_d = nc.dram_tensor("a_pad", [B_LOC, S], bf16, kind="ExternalInput").ap()
    k_d = nc.dram_tensor("k_pad", [B_LOC, S], bf16, kind="ExternalInput").ap()
    v_d = nc.dram_tensor("values_rev", [B_LOC, T], bf16, kind="ExternalInput").ap()
    lp_d = nc.dram_tensor("log_probs_rev", [B_LOC, T], bf16,
                          kind="ExternalInput").ap()
    out_d = nc.dram_tensor("out", [P, N_OUT], f32, kind="ExternalOutput").ap()
    pe_d = nc.dram_tensor("pe_out", [1, PE_N], f32, kind="ExternalOutput").ap()

    a_v = a_d.rearrange("(n p m) s -> n p (m s)", p=P, m=M)
    k_v = k_d.rearrange("(n p m) s -> n p (m s)", p=P, m=M)
    v_v = v_d.rearrange("(n p m) t -> n p (m t)", p=P, m=M)
    lp_v = lp_d.rearrange("(n p m) t -> n p (m t)", p=P, m=M)

    with tile.TileContext(nc) as tc:
        with (
            tc.tile_pool(name="const", bufs=1) as constp,
            tc.tile_pool(name="ins", bufs=3) as ins,
            tc.tile_pool(name="work", bufs=3) as work,
            tc.tile_pool(name="accp", bufs=1) as accp,
            tc.tile_pool(name="psum", bufs=1, space="PSUM") as psp,
        ):
            acc_dve = accp.tile([P, N_DVE], f32)
            acc_act = accp.tile([P, N_ACT], f32)
            ones = constp.tile([P, 1], bf16)
            nc.gpsimd.memset(ones[:], 1.0)
            ps_lp = psp.tile([1, PE_N], f32)

            for n in range(NT):
                a_t = ins.tile([P, FP], bf16)
                k_t = ins.tile([P, FP], bf16)
                v_t = ins.tile([P, F], bf16)
                lp_t = ins.tile([P, F], bf16)
                nc.sync.dma_start(a_t[:], a_v[n])
                nc.sync.dma_start(k_t[:], k_v[n])
                nc.sync.dma_start(v_t[:], v_v[n])
                nc.sync.dma_start(lp_t[:], lp_v[n])

                phi_t = work.tile([P, FP], bf16)
                retm_t = work.tile([P, F], bf16)
                av_t = work.tile([P, F], bf16)
                lnv_t = work.tile([P, F], bf16)
                ar_t = work.tile([P, F], bf16)
                lnr_t = work.tile([P, F], bf16)
                d_t = work.tile([P, F], bf16)
                j1_t = work.tile([P, F], bf16)

                v3 = v_t[:].rearrange("p (m t) -> p m t", t=T)
                phi3 = phi_t[:].rearrange("p (m s) -> p m s", s=S)
                retm3 = retm_t[:].rearrange("p (m t) -> p m t", t=T)
                phi_pay = phi3[:, :, 1:S]  # payload slots, stream order

                # DVE: TD(lambda) scan, one instruction per tile
                nc.vector.tensor_tensor_scan(
                    phi_t[:], k_t[:], a_t[:], 0.0, OP.mult, OP.add
                )
                # DVE: retm = RATIO*v - phi (= -ret), all natural strides
                nc.vector.scalar_tensor_tensor(
                    retm3, v3, RATIO, phi_pay, OP.mult, OP.subtract
                )
                # DVE: extrema of retm (signs fixed on host)
                nc.vector.tensor_reduce(
                    acc_dve[:, NT + n : NT + n + 1], retm_t[:],
                    axis=AX.X, op=OP.max,
                )
                nc.vector.tensor_reduce(
                    acc_dve[:, 2 * NT + n : 2 * NT + n + 1], retm_t[:],
                    axis=AX.X, op=OP.min,
                )
                # DVE: sum(lp * retm) with fp32 accumulator
                nc.vector.scalar_tensor_tensor(
                    j1_t[:], lp_t[:], 1.0, retm_t[:], OP.mult, OP.mult,
                    accum_out=acc_dve[:, n : n + 1],
                )
                # PE: sum(lp) via ones-matmul into PSUM
                for h in range(F // PE_N):
                    nc.tensor.matmul(
                        ps_lp[:], ones[:], lp_t[:, h * PE_N : (h + 1) * PE_N],
                        start=(n == 0 and h == 0),
                        stop=(n == NT - 1 and h == F // PE_N - 1),
                    )
                # ACT: symlog magnitudes (bf16 in/out)
                nc.scalar.activation(av_t[:], v_t[:], AF.Abs)
                nc.scalar.activation(lnv_t[:], av_t[:], AF.Ln, bias=1.0)
                nc.scalar.activation(ar_t[:], retm_t[:], AF.Abs)
                nc.scalar.activation(lnr_t[:], ar_t[:], AF.Ln, bias=1.0)
                # DVE: signed symlogs via uint16 sign-bit copy
                _stt_uint_imm(
                    nc.vector, av_t[:].bitcast(u16), v_t[:].bitcast(u16),
                    0x8000, lnv_t[:].bitcast(u16),
                    OP.bitwise_and, OP.bitwise_or,
                )
                _stt_uint_imm(
                    nc.vector, ar_t[:].bitcast(u16), retm_t[:].bitcast(u16),
                    0x8000, lnr_t[:].bitcast(u16),
                    OP.bitwise_and, OP.bitwise_or,
                )
                # DVE: d = sv + sr' (= symlog v - symlog ret)
                nc.vector.tensor_add(d_t[:], av_t[:], ar_t[:])
                # ACT: critic partial sums (fp32 accumulate)
                nc.scalar.activation(
                    j1_t[:], d_t[:], AF.Square,
                    accum_out=acc_act[:, n : n + 1],
                )

            pe_sb = accp.tile([1, PE_N], f32)
            nc.scalar.copy(pe_sb[:], ps_lp[:])
            nc.sync.dma_start(out_d[:, 0:N_DVE], acc_dve[:])
            nc.sync.dma_start(out_d[:, N_DVE:N_OUT], acc_act[:])
            nc.sync.dma_start(pe_d, pe_sb[:])

    nc.compile()
    return nc


_NC = None


def _get_nc():
    global _NC
    if _NC is None:
        _NC = build_module()
    return _NC


def _run(in_maps, trace=False, **kwargs):
    return run_bass_kernel_spmd(
        _get_nc(), in_maps, core_ids=list(range(NCORES)), trace=trace, **kwargs
    )


def prepare(rewards, values, continues, bootstrap, log_probs, entropy):
    """Host prep: padded reversed bf16 scan streams + reversed bf16 v/lp,
    plus the exact host-side sums that never need the device."""
    r = np.asarray(rewards, dtype=np.float32)
    v = np.asarray(values, dtype=np.float32)
    c = np.asarray(continues, dtype=np.float32)
    bs = np.asarray(bootstrap, dtype=np.float32)
    lp = np.asarray(log_probs, dtype=np.float32)
    en = np.asarray(entropy, dtype=np.float32)

    a_pad = np.empty((B, S), dtype=BF)
    a_pad[:, 0] = (bs * np.float32(1.0 + RATIO)).astype(BF)
    a_pad[:, 1:] = (r + np.float32(RATIO) * v)[:, ::-1].astype(BF)
    k_pad = np.empty((B, S), dtype=BF)
    k_pad[:, 0] = BF(0.0)
    k_pad[:, 1:] = (np.float32(K2) * c)[:, ::-1].astype(BF)
    v_rev = np.ascontiguousarray(v[:, ::-1]).astype(BF)
    lp_rev = np.ascontiguousarray(lp[:, ::-1]).astype(BF)

    host = {
        "u2": np.dot(
            lp.ravel().astype(np.float64), v.ravel().astype(np.float64)
        ),
        "sent": en.sum(dtype=np.float64),
    }

    in_maps = []
    for i in range(NCORES):
        sl = slice(i * B_LOC, (i + 1) * B_LOC)
        in_maps.append(
            {
                "a_pad": np.ascontiguousarray(a_pad[sl]),
                "k_pad": np.ascontiguousarray(k_pad[sl]),
                "values_rev": np.ascontiguousarray(v_rev[sl]),
                "log_probs_rev": np.ascontiguousarray(lp_rev[sl]),
            }
        )
    return in_maps, host


def combine(results, host):
    outs = np.stack([res["out"] for res in results]).astype(np.float64)
    pe = np.stack([res["pe_out"] for res in results]).astype(np.float64)
    u1 = -outs[:, :, 0:NT].sum()             # sum lp*ret
    mn = -outs[:, :, NT : 2 * NT].max()      # min ret
    mx = -outs[:, :, 2 * NT : 3 * NT].min()  # max ret
    d2 = outs[:, :, N_DVE:].sum()
    slp = pe.sum()
    u2 = host["u2"]
    sent = host["sent"]

    n = float(B * T)
    ema = 1.0 - RETURN_EMA_DECAY
    lo_n = ema * mn
    hi_n = 1.0 + ema * (mx - 1.0)
    scale = max(hi_n - lo_n, 1.0)
    pg = -((u1 / n) / scale - lo_n * (slp / n) / scale - (u2 / n))
    entropy_loss = -ENTROPY_SCALE * (sent / n)
    critic = d2 / n
    return np.float32(pg + entropy_loss + critic)


def kernel(rewards, values, continues, bootstrap, log_probs, entropy):
    in_maps, host = prepare(
        rewards, values, continues, bootstrap, log_probs, entropy
    )
    results = _run(in_maps).results
    return combine(results, host)


# revision 12
# speedup vs baseline: 1.7584x; 1.0578x over previous
"""ActorCriticLoss (TD-lambda + symlog critic) on 8 Trainium2 NeuronCores.

Data-parallel over the batch axis (65536 -> 8 x 8192). The device reduces
each shard to per-partition partials; the O(1) loss assembly runs on the
host in float64.

Math: with phi_t = ret_t + (K1/K2) v_t the TD(lambda) recurrence becomes
  phi_t = a_t + K2 c_t phi_{t+1},   a_t = r_t + (K1/K2) v_t
(the c*v_next product cancels). The backward scan runs as one forward
`tensor_tensor_scan` per tile over per-row padded, time-reversed streams
([pad, t=63..0] per row, k_pad = 0, a_pad = bootstrap*(1+K1/K2)) so the
fp32 scan carry reinitializes at every row boundary. The device works with
retm = -ret throughout; signs are fixed on the host.

Split of labor (driven by measured per-instruction HW costs):
 - HOST (numpy, exact fp32/f64): builds the padded bf16 a/k streams and
   time-reversed bf16 v/log_probs; computes sum(entropy) and sum(lp*v)
   from the fp32 originals (entropy/rewards/continues never hit the device
   elementwise paths they aren't needed in).
 - DVE (all-bf16, 2x rate; fp32 scan carry + fp32 accumulators): scan,
   retm, sum(lp*retm), sign-copies via uint16 bit ops, d, min/max.
 - ACT: |v|, ln(1+|v|), |ret|, ln(1+|ret|), v->bf16, sum(d^2) via
   Square+accum (bf16 in, fp32 accumulate).
 - PE: sum(lp) via ones-matmul into PSUM.
"""

import sys

import ml_dtypes
import numpy as np

sys.path.insert(0, "/opt/trn_rl_repo")

import concourse.bass as bass  # noqa: E402
import concourse.mybir as mybir  # noqa: E402
import concourse.tile as tile  # noqa: E402
from concourse import bacc  # noqa: E402
from concourse.bass_utils import run_bass_kernel_spmd  # noqa: E402

B, T = 65536, 64
NCORES = 8
B_LOC = B // NCORES
P = 128
M = 16                       # rows per partition per tile
NT = B_LOC // (P * M)
F = M * T                    # payload elements/partition per tile
S = T + 1                    # padded slots per row
FP = M * S

DISCOUNT, LAMBDA = 0.997, 0.95
ENTROPY_SCALE = 0.0003
RETURN_EMA_DECAY = 0.99
K2 = DISCOUNT * LAMBDA
RATIO = (1.0 - LAMBDA) / LAMBDA

f32 = mybir.dt.float32
bf16 = mybir.dt.bfloat16
u16 = mybir.dt.uint16
AX = mybir.AxisListType
OP = mybir.AluOpType
AF = mybir.ActivationFunctionType
BF = ml_dtypes.bfloat16

# acc_dve columns: mxm(0:NT) mnm(NT:2NT); acc_act: d2(0:NT)
N_DVE = 2 * NT
N_ACT = NT
N_OUT = N_DVE + N_ACT
PE_N = 512


def _stt_uint_imm(eng, out, in0, imm, in1, op0, op1, imm_dtype=u16,
                  accum_out=None):
    """scalar_tensor_tensor with an integer-typed immediate (the public
    wrapper emits float32 immediates; bitvec ops need the immediate typed
    like src/dst)."""
    outs = [eng.lower_ap(out)]
    if accum_out is not None:
        outs.append(eng.lower_ap(accum_out))
    return eng.add_instruction(
        mybir.InstTensorScalarPtr(
            name=eng.bass.get_next_instruction_name(),
            is_scalar_tensor_tensor=True,
            op0=op0,
            op1=op1,
            ins=[
                eng.lower_ap(in0),
                mybir.ImmediateValue(dtype=imm_dtype, value=imm),
                eng.lower_ap(in1),
            ],
            outs=outs,
        )
    )


def build_module():
    nc = bacc.Bacc(
        "TRN2", target_bir_lowering=False, debug=False, enable_asserts=False
    )
    a_d = nc.dram_tensor("a_pad", [B_LOC, S], bf16, kind="ExternalInput").ap()
    k_d = nc.dram_tensor("k_pad", [B_LOC, S], bf16, kind="ExternalInput").ap()
    v_d = nc.dram_tensor("vs_rev", [B_LOC, T], bf16, kind="ExternalInput").ap()
    lp_d = nc.dram_tensor("log_probs_rev", [B_LOC, T], bf16,
                          kind="ExternalInput").ap()
    out_d = nc.dram_tensor("out", [P, N_OUT], f32, kind="ExternalOutput").ap()
    pe_d = nc.dram_tensor("pe_out", [1, 2 * PE_N], f32, kind="ExternalOutput").ap()

    a_v = a_d.rearrange("(n p m) s -> n p (m s)", p=P, m=M)
    k_v = k_d.rearrange("(n p m) s -> n p (m s)", p=P, m=M)
    v_v = v_d.rearrange("(n p m) t -> n p (m t)", p=P, m=M)
    lp_v = lp_d.rearrange("(n p m) t -> n p (m t)", p=P, m=M)

    with tile.TileContext(nc) as tc:
        with (
            tc.tile_pool(name="const", bufs=1) as constp,
            tc.tile_pool(name="ins", bufs=3) as ins,
            tc.tile_pool(name="work", bufs=3) as work,
            tc.tile_pool(name="accp", bufs=1) as accp,
            tc.tile_pool(name="psum", bufs=1, space="PSUM") as psp,
        ):
            acc_dve = accp.tile([P, N_DVE], f32)
            acc_act = accp.tile([P, N_ACT], f32)
            ones = constp.tile([P, 1], bf16)
            nc.gpsimd.memset(ones[:], 1.0)
            ps_lp = psp.tile([1, PE_N], f32)
            ps_u1 = psp.tile([1, PE_N], f32)

            for n in range(NT):
                a_t = ins.tile([P, FP], bf16)
                k_t = ins.tile([P, FP], bf16)
                v_t = ins.tile([P, F], bf16)
                lp_t = ins.tile([P, F], bf16)
                nc.sync.dma_start(a_t[:], a_v[n])
                nc.sync.dma_start(k_t[:], k_v[n])
                nc.sync.dma_start(v_t[:], v_v[n])
                nc.sync.dma_start(lp_t[:], lp_v[n])

                phi_t = work.tile([P, FP], bf16)
                retm_t = work.tile([P, F], bf16)
                av_t = work.tile([P, F], bf16)
                lnv_t = work.tile([P, F], bf16)
                ar_t = work.tile([P, F], bf16)
                lnr_t = work.tile([P, F], bf16)
                d_t = work.tile([P, F], bf16)
                j1_t = work.tile([P, F], bf16)

                v3 = v_t[:].rearrange("p (m t) -> p m t", t=T)
                phi3 = phi_t[:].rearrange("p (m s) -> p m s", s=S)
                retm3 = retm_t[:].rearrange("p (m t) -> p m t", t=T)
                phi_pay = phi3[:, :, 1:S]  # payload slots, stream order

                # DVE: TD(lambda) scan, one instruction per tile
                nc.vector.tensor_tensor_scan(
                    phi_t[:], k_t[:], a_t[:], 0.0, OP.mult, OP.add
                )
                # DVE: retm = vs - phi (= -ret); vs = RATIO*v from host
                nc.vector.tensor_sub(retm3, v3, phi_pay)
                # DVE: extrema of retm (signs fixed on host)
                nc.vector.tensor_reduce(
                    acc_dve[:, n : n + 1], retm_t[:],
                    axis=AX.X, op=OP.max,
                )
                nc.vector.tensor_reduce(
                    acc_dve[:, NT + n : NT + n + 1], retm_t[:],
                    axis=AX.X, op=OP.min,
                )
                # DVE: lp*retm product; PE: its sum + sum(lp)
                nc.vector.tensor_mul(j1_t[:], lp_t[:], retm_t[:])
                for h in range(F // PE_N):
                    first = n == 0 and h == 0
                    last = n == NT - 1 and h == F // PE_N - 1
                    sl = slice(h * PE_N, (h + 1) * PE_N)
                    nc.tensor.matmul(
                        ps_lp[:], ones[:], lp_t[:, sl], start=first, stop=last
                    )
                    nc.tensor.matmul(
                        ps_u1[:], ones[:], j1_t[:, sl], start=first, stop=last
                    )
                # ACT: symlog magnitudes (bf16 in/out)
                nc.scalar.activation(av_t[:], v_t[:], AF.Abs, scale=1.0 / RATIO)
                nc.scalar.activation(lnv_t[:], av_t[:], AF.Ln, bias=1.0)
                nc.scalar.activation(ar_t[:], retm_t[:], AF.Abs)
                nc.scalar.activation(lnr_t[:], ar_t[:], AF.Ln, bias=1.0)
                # DVE: signed symlogs via uint16 sign-bit copy
                _stt_uint_imm(
                    nc.vector, av_t[:].bitcast(u16), v_t[:].bitcast(u16),
                    0x8000, lnv_t[:].bitcast(u16),
                    OP.bitwise_and, OP.bitwise_or,
                )
                _stt_uint_imm(
                    nc.vector, ar_t[:].bitcast(u16), retm_t[:].bitcast(u16),
                    0x8000, lnr_t[:].bitcast(u16),
                    OP.bitwise_and, OP.bitwise_or,
                )
                # DVE: d = sv + sr' (= symlog v - symlog ret)
                nc.vector.tensor_add(d_t[:], av_t[:], ar_t[:])
                # ACT: critic partial sums (fp32 accumulate)
                nc.scalar.activation(
                    j1_t[:], d_t[:], AF.Square,
                    accum_out=acc_act[:, n : n + 1],
                )

            pe_sb = accp.tile([1, 2 * PE_N], f32)
            nc.scalar.copy(pe_sb[:, 0:PE_N], ps_lp[:])
            nc.scalar.copy(pe_sb[:, PE_N:], ps_u1[:])
            nc.sync.dma_start(out_d[:, 0:N_DVE], acc_dve[:])
            nc.sync.dma_start(out_d[:, N_DVE:N_OUT], acc_act[:])
            nc.sync.dma_start(pe_d, pe_sb[:])

    nc.compile()
    return nc


_NC = None


def _get_nc():
    global _NC
    if _NC is None:
        _NC = build_module()
    return _NC


def _run(in_maps, trace=False, **kwargs):
    return run_bass_kernel_spmd(
        _get_nc(), in_maps, core_ids=list(range(NCORES)), trace=trace, **kwargs
    )


def prepare(rewards, values, continues, bootstrap, log_probs, entropy):
    """Host prep: padded reversed bf16 scan streams + reversed bf16 v/lp,
    plus the exact host-side sums that never need the device."""
    r = np.asarray(rewards, dtype=np.float32)
    v = np.asarray(values, dtype=np.float32)
    c = np.asarray(continues, dtype=np.float32)
    bs = np.asarray(bootstrap, dtype=np.float32)
    lp = np.asarray(log_probs, dtype=np.float32)
    en = np.asarray(entropy, dtype=np.float32)

    a_pad = np.empty((B, S), dtype=BF)
    a_pad[:, 0] = (bs * np.float32(1.0 + RATIO)).astype(BF)
    a_pad[:, 1:] = (r + np.float32(RATIO) * v)[:, ::-1].astype(BF)
    k_pad = np.empty((B, S), dtype=BF)
    k_pad[:, 0] = BF(0.0)
    k_pad[:, 1:] = (np.float32(K2) * c)[:, ::-1].astype(BF)
    vs_rev = np.ascontiguousarray((np.float32(RATIO) * v)[:, ::-1]).astype(BF)
    lp_rev = np.ascontiguousarray(lp[:, ::-1]).astype(BF)

    host = {
        "u2": np.dot(
            lp.ravel().astype(np.float64), v.ravel().astype(np.float64)
        ),
        "sent": en.sum(dtype=np.float64),
    }

    in_maps = []
    for i in range(NCORES):
        sl = slice(i * B_LOC, (i + 1) * B_LOC)
        in_maps.append(
            {
                "a_pad": np.ascontiguousarray(a_pad[sl]),
                "k_pad": np.ascontiguousarray(k_pad[sl]),
                "vs_rev": np.ascontiguousarray(vs_rev[sl]),
                "log_probs_rev": np.ascontiguousarray(lp_rev[sl]),
            }
        )
    return in_maps, host


def combine(results, host):
    outs = np.stack([res["out"] for res in results]).astype(np.float64)
    pe = np.stack([res["pe_out"] for res in results]).astype(np.float64)
    mn = -outs[:, :, 0:NT].max()             # min ret
    mx = -outs[:, :, NT : 2 * NT].min()      # max ret
    d2 = outs[:, :, N_DVE:].sum()
    slp = pe[:, 0, 0:PE_N].sum()
    u1 = -pe[:, 0, PE_N:].sum()              # sum lp*ret
    u2 = host["u2"]
    sent = host["sent"]

    n = float(B * T)
    ema = 1.0 - RETURN_EMA_DECAY
    lo_n = ema * mn
    hi_n = 1.0 + ema * (mx - 1.0)
    scale = max(hi_n - lo_n, 1.0)
    pg = -((u1 / n) / scale - lo_n * (slp / n) / scale - (u2 / n))
    entropy_loss = -ENTROPY_SCALE * (sent / n)
    critic = d2 / n
    return np.float32(pg + entropy_loss + critic)


def kernel(rewards, values, continues, bootstrap, log_probs, entropy):
    in_maps, host = prepare(
        rewards, values, continues, bootstrap, log_probs, entropy
    )
    results = _run(in_maps).results
    return combine(results, host)
